# revision 1
# baseline (speedup 1.0000x reference)
"""SoftRas-style soft rasterizer on 8 Trainium2 NeuronCores.

Strategy:
- All per-(face,pixel) affine quantities (barycentric w0/w1, edge projections
  u/l2, squared vertex distances) are produced by TensorE matmuls against the
  pixel basis [1, px, py, px^2+py^2] (K=4).
- The nonlinear chain (clip/sqrt/sigmoid/reciprocal/exp/log) runs on
  VectorE/ScalarE with faces on partitions (128/chunk), pixels on the free dim.
- Per-pixel max over faces (softmax shift) via PE transposes + free-dim max.
- Face-direction reductions (rgb accumulation, dsum, sum log(1-p)) via PE
  matmuls against textures / ones.
- Host (numpy): per-face coefficient prep, per-tile face culling, per-core
  load balancing (every core gets an identical chunk-count pattern so one
  SPMD program serves all 8 cores), final divide + alpha exponentiation.
"""
import sys
sys.path.insert(0, '/opt/trn_rl_repo')
import numpy as np
import ml_dtypes
from contextlib import ExitStack

import concourse.bass as bass
import concourse.bacc as bacc
import concourse.tile as tile
import concourse.mybir as mybir
from concourse.bass_utils import run_bass_kernel_spmd
from concourse.masks import make_identity

TRACE = False
LAST_RESULT = None

F_TOT = 512
H = W = 256
NCORES = 8
TP = 512              # pixels per tile position (2 image rows)
NPOS = (H * W) // (NCORES * TP)   # 16 tile positions per core
SIGMA = 1e-2
GAMMA = 1e-3
EPS = 1e-3
NEAR, FAR = 1.0, 100.0
FP = mybir.dt.float32
F32R = mybir.dt.float32r
BF = mybir.dt.bfloat16
AL = mybir.AluOpType
AF = mybir.ActivationFunctionType


def _host_prep(face_vertices):
    """Per-face coefficients in basis [1, px, py, r2], fp64 -> fp32."""
    fv = np.asarray(face_vertices, np.float64)[0]          # [F,3,3]
    F = fv.shape[0]
    x = fv[:, :, 0]; y = fv[:, :, 1]; z = fv[:, :, 2]
    x0, x1, x2 = x[:, 0], x[:, 1], x[:, 2]
    y0, y1, y2 = y[:, 0], y[:, 1], y[:, 2]

    den = (y1 - y2) * (x0 - x2) + (x2 - x1) * (y0 - y2)
    den = np.where(np.abs(den) < 1e-10, 1e-10, den)
    W0c = np.stack([(-(y1 - y2) * x2 - (x2 - x1) * y2) / den,
                    (y1 - y2) / den, (x2 - x1) / den, np.zeros(F)], -1)
    W1c = np.stack([(-(y2 - y0) * x2 - (x0 - x2) * y2) / den,
                    (y2 - y0) / den, (x0 - x2) / den, np.zeros(F)], -1)

    anchors = [(x0, y0), (x1, y1), (x2, y2)]
    pairs = [(0, 1), (1, 2), (2, 0)]
    # per edge: U = ((p-a).d)/|d| (along-line coord), LD = cross(p-a, d)/|d|
    # (signed line distance). d2seg = LD^2 + max(|U - L/2| - L/2, 0)^2.
    UT = np.zeros((3, F, 4)); S2 = np.zeros((3, F, 4)); HL = np.zeros((3, F))
    for e, (ia, ib) in enumerate(pairs):
        ax, ay = anchors[ia]; bx, by = anchors[ib]
        dx, dy = bx - ax, by - ay
        L = np.sqrt(np.maximum(dx * dx + dy * dy, 1e-12))
        iL = 1.0 / L
        UT[e, :, 0] = (-ax * dx - ay * dy) * iL - L / 2.0   # y = U - L/2
        UT[e, :, 1] = dx * iL
        UT[e, :, 2] = dy * iL
        S2[e, :, 0] = (ay * dx - ax * dy) * iL
        S2[e, :, 1] = dy * iL
        S2[e, :, 2] = -dx * iL
        HL[e] = L / 2.0
    iz = 1.0 / z
    zmin = z.min(1); zmax = z.max(1)
    assert z.min() > NEAR + 0.05 and z.max() < FAR - 0.05, \
        "kernel fast path assumes all vertex depths strictly inside (NEAR,FAR)"
    return dict(W0c=W0c, W1c=W1c, UT=UT, S2=S2, HL=HL, iz=iz,
                ymin=y.min(1), ymax=y.max(1), xmin=x.min(1), xmax=x.max(1),
                zmin=zmin, zmax=zmax)


def _cull_and_balance(prep):
    """Per tile (4 rows x 128 px), the kept-face list; balanced so all cores
    share one chunk-count pattern. Returns (pattern, assign) where
    assign[core][pos] = (tile_index, face_index_array padded with -1)."""
    nyb = H // 4
    pixc = ((np.arange(H) + 0.5) / H) * 2.0 - 1.0
    tiles = []
    for yb in range(nyb):
        for xb in range(2):
            tiles.append((pixc[4 * yb], pixc[4 * yb + 3],
                          pixc[128 * xb], pixc[128 * xb + 127]))
    tiles = np.array(tiles)                                # [nb, 4]
    nb = len(tiles)
    ygap = np.maximum(0.0, np.maximum(
        prep['ymin'][None, :] - tiles[:, 1:2],
        tiles[:, 0:1] - prep['ymax'][None, :]))
    xgap = np.maximum(0.0, np.maximum(
        prep['xmin'][None, :] - tiles[:, 3:4],
        tiles[:, 2:3] - prep['xmax'][None, :]))
    gap = np.sqrt(xgap ** 2 + ygap ** 2)
    znUB = (FAR - prep['zmin']) / (FAR - NEAR)
    znLB = (FAR - prep['zmax']) / (FAR - NEAR)
    D = znLB.max()
    MH = znUB.max()
    # every pixel's true zmax lies in [D, MH]; if that window is narrow a
    # single global softmax shift MH is exact (no over/underflow possible)
    assert MH - D <= 0.07, "global-shift fast path needs a narrow zmax window"
    keep = (gap < 0.17) | ((gap / SIGMA) + (D - znUB) / GAMMA < 87.0)  # [nb,F]

    counts = np.maximum(1, np.ceil(keep.sum(1) / 128).astype(int))
    order = np.argsort(-counts, kind='stable')             # bands, desc count
    pattern = [int(counts[order[p * NCORES]]) for p in range(NPOS)]
    assign = [[None] * NPOS for _ in range(NCORES)]
    for p in range(NPOS):
        for c in range(NCORES):
            b = int(order[p * NCORES + c])
            faces = np.nonzero(keep[b])[0]
            pad = pattern[p] * 128 - len(faces)
            assert pad >= 0
            faces = np.concatenate([faces, -np.ones(pad, np.int64)])
            assign[c][p] = (b, faces)
    return pattern, assign, float(max(MH, EPS))


# 3-way bf16 split: x = h + m + l with each part bf16-exact. Products of
# bf16-exact values are exact in the PE's f32r mode, so a 6-combo expansion
# (dropping <1e-7 cross terms) gives fp32-class precision at full PE rate.
COMBOS = [(0, 0), (0, 1), (1, 0), (0, 2), (1, 1), (2, 0)]
NK = 4 * len(COMBOS)


def _split3(a):
    a = np.asarray(a, np.float64)
    h = a.astype(ml_dtypes.bfloat16).astype(np.float64)
    r = a - h
    m = r.astype(ml_dtypes.bfloat16).astype(np.float64)
    l = (r - m).astype(ml_dtypes.bfloat16).astype(np.float64)
    return [h, m, l]


def _face_arrays(prep, textures, faces):
    """Pack per-chunk coefficient/texture/scalar arrays for one chunk of 128
    face slots (index -1 = inert dummy)."""
    f = np.asarray(faces)
    dummy = f < 0
    fi = np.where(dummy, 0, f)

    def D(a):  # zero out dummies
        a = np.asarray(a, np.float64).copy()
        a[dummy] = 0.0
        return a

    # quantity order: U01,LD01,U12,LD12,U20,LD20,W0,W1,W2 -> coef[4, 9, 128]
    coef = np.zeros((4, 9, 128))
    for e in range(3):
        coef[:, 2 * e, :] = D(prep['UT'][e][fi]).T
        coef[:, 2 * e + 1, :] = D(prep['S2'][e][fi]).T
    coef[:, 6, :] = D(prep['W0c'][fi]).T
    coef[:, 7, :] = D(prep['W1c'][fi]).T
    # dummies: W0=W1=-1 (outside, wc2=1), LD=10 (dist 10 -> prob 0),
    # iz=0.011 -> zp~90.9 -> zn~0.092 (never the argmax), halfL=0.5
    coef[0, 1, dummy] = 10.0
    coef[0, 3, dummy] = 10.0
    coef[0, 5, dummy] = 10.0
    coef[0, 6, dummy] = -1.0
    coef[0, 7, dummy] = -1.0
    coef[:, 8, :] = -coef[:, 6, :] - coef[:, 7, :]
    coef[0, 8, :] += 1.0                                   # w2 = 1 - w0 - w1
    cs = _split3(coef)
    coefk = np.zeros((NK, 9, 128), np.float32)
    for t, (ci, bi) in enumerate(COMBOS):
        coefk[4 * t:4 * t + 4] = cs[ci].astype(np.float32)

    tex = np.asarray(textures, np.float64)[0][fi]          # [128,3,3] (k,c)
    tex[dummy] = 0.0

    scal = np.zeros((128, 9))
    izf = prep['iz'][fi]
    izf[dummy] = 0.011
    scal[:, 0:3] = izf
    hlf = prep['HL'][:, fi].T
    hlf[dummy] = 0.5
    scal[:, 3:6] = hlf
    scal[:, 6:9] = -hlf
    return coefk, tex, scal


def _build_program(pattern, mhat):
    """One SPMD Bass program; chunk counts per position given by pattern."""
    totc = sum(pattern)
    kmax = max(pattern)
    nc = bacc.Bacc("TRN2", target_bir_lowering=False, debug=False,
                   num_devices=NCORES)
    d_coef = nc.dram_tensor("coef", [totc, 24, 9 * 128], F32R, kind="ExternalInput")
    d_basis = nc.dram_tensor("basis", [NPOS, 24, TP], F32R, kind="ExternalInput")
    d_tex = nc.dram_tensor("tex", [128, totc * 9], FP, kind="ExternalInput")
    d_scal = nc.dram_tensor("scal", [128, totc * 9], FP, kind="ExternalInput")
    d_out = nc.dram_tensor("out6", [5, NPOS * TP], FP, kind="ExternalOutput")

    with ExitStack() as ctx:
        tc = ctx.enter_context(tile.TileContext(nc))
        const = ctx.enter_context(tc.tile_pool(name="const", bufs=1))
        stage = ctx.enter_context(tc.tile_pool(name="stage", bufs=3))
        basp = ctx.enter_context(tc.tile_pool(name="basp", bufs=3))
        work = ctx.enter_context(tc.tile_pool(name="work", bufs=2))
        store = ctx.enter_context(tc.tile_pool(name="store", bufs=3))
        zm = ctx.enter_context(tc.tile_pool(name="zm", bufs=3))
        qp = ctx.enter_context(tc.tile_pool(name="qp", bufs=6, space="PSUM"))
        accp = ctx.enter_context(tc.tile_pool(name="accp", bufs=2, space="PSUM"))

        onesc = const.tile([128, 1], BF)
        nc.vector.memset(onesc, 1.0)
        onesf = const.tile([128, 1], FP)
        nc.vector.memset(onesf, 1.0)
        b_sqrt = const.tile([128, 1], FP)
        nc.vector.memset(b_sqrt, 1e-12)
        b_ln = const.tile([128, 1], FP)
        nc.vector.memset(b_ln, 1e-30)
        b_exp = const.tile([128, 1], FP)
        nc.vector.memset(b_exp, -mhat / GAMMA)
        tex_sb = const.tile([128, totc * 9], FP)
        nc.sync.dma_start(out=tex_sb, in_=d_tex[:, :])
        scal_sb = const.tile([128, totc * 9], FP)
        nc.sync.dma_start(out=scal_sb, in_=d_scal[:, :])

        jj = 0
        base = [0]
        for p in pattern:
            base.append(base[-1] + p)
        for pp in range(0, NPOS, 2):
            pair = [pp, pp + 1]
            st8 = {}
            for pos in pair:
                K = pattern[pos]
                bas = basp.tile([24, TP], F32R, tag="bas")
                nc.sync.dma_start(out=bas, in_=d_basis[pos, :, :])

                zn_st = store.tile([128, kmax, TP], FP, tag="zn_st")
                pv_st = store.tile([128, kmax, TP], FP, tag="pv_st")
                wn_st = [store.tile([128, kmax, TP], FP, tag=f"wn{k}_st",
                                     name=f"wn{k}_st") for k in range(3)]
                d2_st = store.tile([128, kmax, TP], FP, tag="d2_st")
                sb_st = store.tile([128, kmax, TP], mybir.dt.uint32,
                                   tag="sb_st")
                acc = accp.tile([65, TP], FP, tag="acc")
                st8[pos] = (zn_st, pv_st, wn_st, d2_st, sb_st, acc)

                # -- phase 1a: per chunk, everything except sqrt/sig/ln/exp --
                for j in range(K):
                    cj = base[pos] + j
                    st = stage.tile([24, 9 * 128], F32R, tag="st")
                    nc.sync.dma_start(out=st, in_=d_coef[cj, :, :])
                    q = [qp.tile([128, TP], FP, tag="q", name=f"q{qi}")
                         for qi in range(9)]
                    for qi in range(9):
                        nc.tensor.matmul(q[qi],
                                         st[:, qi * 128:(qi + 1) * 128],
                                         bas, start=True, stop=True)
                    sc = lambda i: scal_sb[:, cj * 9 + i: cj * 9 + i + 1]

                    # d2_e = LD^2 + relu(|U - L/2| - L/2)^2 (U-L/2 from PE)
                    ru = [work.tile([128, TP], FP, tag=f"ru{e}",
                                    name=f"ru{e}") for e in range(3)]
                    tt = [work.tile([128, TP], FP, tag=f"t{e}",
                                    name=f"t{e}") for e in range(3)]
                    for e in range(3):
                        nc.scalar.activation(ru[e], q[2 * e + 1], AF.Square)
                        nc.scalar.activation(tt[e], q[2 * e], AF.Abs)
                        nc.scalar.activation(tt[e], tt[e], AF.Relu,
                                             bias=sc(6 + e))    # overshoot
                        nc.scalar.activation(tt[e], tt[e], AF.Square)
                        nc.vector.tensor_tensor(out=ru[e], in0=ru[e],
                                                in1=tt[e], op=AL.add)
                    nc.vector.tensor_tensor(out=ru[0], in0=ru[0], in1=ru[1],
                                            op=AL.min)
                    nc.vector.tensor_tensor(out=d2_st[:, j, :], in0=ru[0],
                                            in1=ru[2], op=AL.min)

                    cw0 = work.tile([128, TP], FP, tag="cw0")
                    nc.scalar.activation(cw0, q[6], AF.Copy)
                    cw1 = work.tile([128, TP], FP, tag="cw1")
                    nc.scalar.activation(cw1, q[7], AF.Copy)
                    cw2 = work.tile([128, TP], FP, tag="cw2")
                    nc.scalar.activation(cw2, q[8], AF.Copy)
                    m1 = work.tile([128, TP], FP, tag="m1")
                    nc.vector.tensor_tensor(out=m1, in0=cw0, in1=cw1,
                                            op=AL.min)
                    nc.vector.tensor_tensor(out=m1, in0=m1, in1=cw2,
                                            op=AL.min)
                    nc.vector.tensor_scalar(out=sb_st[:, j, :],
                                            in0=m1.bitcast(mybir.dt.uint32),
                                            scalar1=0x80000000, scalar2=None,
                                            op0=AL.bitwise_and)

                    wc0, wc1, wc2 = cw0, cw1, cw2
                    nc.vector.tensor_scalar(out=wc0, in0=cw0, scalar1=0.0,
                                            scalar2=1.0, op0=AL.max,
                                            op1=AL.min)
                    nc.vector.tensor_scalar(out=wc1, in0=cw1, scalar1=0.0,
                                            scalar2=1.0, op0=AL.max,
                                            op1=AL.min)
                    nc.vector.tensor_scalar(out=wc2, in0=cw2, scalar1=0.0,
                                            scalar2=1.0, op0=AL.max,
                                            op1=AL.min)
                    s01 = work.tile([128, TP], FP, tag="s01")
                    nc.vector.tensor_tensor(out=s01, in0=wc0, in1=wc1,
                                            op=AL.add)
                    nc.vector.tensor_tensor(out=s01, in0=s01, in1=wc2,
                                            op=AL.add)
                    invs = work.tile([128, TP], FP, tag="invs")
                    nc.vector.reciprocal_approx_fast(out=invs, in_=s01)
                    r1 = work.tile([128, TP], FP, tag="r1")
                    nc.vector.tensor_scalar(out=r1, in0=wc0, scalar1=sc(0),
                                            scalar2=None, op0=AL.mult)
                    nc.vector.scalar_tensor_tensor(out=r1, in0=wc1,
                                                   scalar=sc(1), in1=r1,
                                                   op0=AL.mult, op1=AL.add)
                    nc.vector.scalar_tensor_tensor(out=r1, in0=wc2,
                                                   scalar=sc(2), in1=r1,
                                                   op0=AL.mult, op1=AL.add)
                    nc.vector.tensor_tensor(out=r1, in0=r1, in1=invs,
                                            op=AL.mult)
                    nc.vector.reciprocal_approx_fast(out=s01, in_=r1)  # zp
                    nc.vector.tensor_scalar(out=zn_st[:, j, :], in0=s01,
                                            scalar1=-1.0 / (FAR - NEAR),
                                            scalar2=FAR / (FAR - NEAR),
                                            op0=AL.mult, op1=AL.add)
                    for k, wck in enumerate([wc0, wc1, wc2]):
                        nc.gpsimd.tensor_tensor(out=wn_st[k][:, j, :],
                                                in0=wck, in1=invs,
                                                op=AL.mult)

            # -- phase 1b over the PAIR: one table load per LUT fn serves 2
            for pos in pair:
                zn_st, pv_st, wn_st, d2_st, sb_st, acc = st8[pos]
                for j in range(pattern[pos]):
                    nc.scalar.activation(d2_st[:, j, :], d2_st[:, j, :],
                                         AF.Sqrt, bias=b_sqrt)
            for pos in pair:
                zn_st, pv_st, wn_st, d2_st, sb_st, acc = st8[pos]
                for j in range(pattern[pos]):
                    nc.vector.tensor_tensor(
                        out=d2_st[:, j, :].bitcast(mybir.dt.uint32),
                        in0=d2_st[:, j, :].bitcast(mybir.dt.uint32),
                        in1=sb_st[:, j, :], op=AL.bitwise_or)
            for pos in pair:
                zn_st, pv_st, wn_st, d2_st, sb_st, acc = st8[pos]
                for j in range(pattern[pos]):
                    nc.scalar.activation(pv_st[:, j, :], d2_st[:, j, :],
                                         AF.Sigmoid, scale=1.0 / SIGMA)
            for pos in pair:
                zn_st, pv_st, wn_st, d2_st, sb_st, acc = st8[pos]
                for j in range(pattern[pos]):
                    q1m = work.tile([128, TP], FP, tag="q1m")
                    nc.vector.tensor_scalar(out=q1m, in0=pv_st[:, j, :],
                                            scalar1=-1.0, scalar2=1.0,
                                            op0=AL.mult, op1=AL.add)
                    lq = work.tile([128, TP], FP, tag="lq")
                    nc.scalar.activation(lq, q1m, AF.Ln, bias=b_ln)
                    nc.tensor.matmul(acc[64:65, :], onesf[:, 0:1], lq,
                                     start=(j == 0),
                                     stop=(j == pattern[pos] - 1))

            # -- phase 3 over the pair: exp weights, rgb/dsum accumulation --
            for pos in pair:
                zn_st, pv_st, wn_st, d2_st, sb_st, acc = st8[pos]
                K = pattern[pos]
                for j in range(K):
                    cj = base[pos] + j
                    d = work.tile([128, TP], FP, tag="d")
                    nc.scalar.activation(d, zn_st[:, j, :], AF.Exp,
                                         scale=1.0 / GAMMA, bias=b_exp)
                    nc.gpsimd.tensor_tensor(out=d, in0=pv_st[:, j, :],
                                            in1=d, op=AL.mult)     # wexp
                    for k in range(3):
                        g = work.tile([128, TP], FP, tag="g", bufs=3)
                        nc.gpsimd.tensor_tensor(out=g, in0=d,
                                                in1=wn_st[k][:, j, :],
                                                op=AL.mult)
                        nc.tensor.matmul(
                            acc[0:3, :],
                            tex_sb[:, cj * 9 + k * 3: cj * 9 + (k + 1) * 3],
                            g, start=(j == 0 and k == 0),
                            stop=(j == K - 1 and k == 2))
                    nc.tensor.matmul(acc[32:33, :], onesf[:, 0:1], d,
                                     start=(j == 0), stop=(j == K - 1))

                o6 = zm.tile([65, TP], FP, tag="o6")
                nc.vector.tensor_copy(o6[0:3, :], acc[0:3, :])
                nc.vector.tensor_copy(o6[32:33, :], acc[32:33, :])
                nc.scalar.activation(o6[64:65, :], acc[64:65, :], AF.Copy)
                nc.sync.dma_start(out=d_out[0:3, pos * TP:(pos + 1) * TP],
                                  in_=o6[0:3, :])
                nc.sync.dma_start(out=d_out[3:4, pos * TP:(pos + 1) * TP],
                                  in_=o6[32:33, :])
                nc.sync.dma_start(out=d_out[4:5, pos * TP:(pos + 1) * TP],
                                  in_=o6[64:65, :])
    nc.compile()
    return nc


def kernel(face_vertices, face_textures):
    prep = _host_prep(face_vertices)
    pattern, assign, mhat = _cull_and_balance(prep)
    totc = sum(pattern)

    pix = ((np.arange(H, dtype=np.float64) + 0.5) / H) * 2.0 - 1.0
    in_maps = []
    for c in range(NCORES):
        coef = np.zeros((totc, NK, 9 * 128), np.float32)
        tex = np.zeros((128, totc * 9), np.float32)
        scal = np.zeros((128, totc * 9), np.float32)
        basis = np.zeros((NPOS, NK, TP), np.float32)
        jj = 0
        for pos in range(NPOS):
            b, faces = assign[c][pos]
            yb, xb = b // 2, b % 2
            py = np.repeat(pix[4 * yb:4 * yb + 4], 128)
            px = np.tile(pix[128 * xb:128 * xb + 128], 4)
            b4 = np.stack([np.ones(TP), px, py, px ** 2 + py ** 2])
            bs = _split3(b4)
            for t, (ci, bi) in enumerate(COMBOS):
                basis[pos, 4 * t:4 * t + 4] = bs[bi].astype(np.float32)
            for j in range(pattern[pos]):
                cf, tx, sl = _face_arrays(prep, face_textures,
                                          faces[j * 128:(j + 1) * 128])
                coef[jj] = cf.reshape(NK, 9 * 128)
                tex[:, jj * 9:(jj + 1) * 9] = tx.reshape(128, 9)
                scal[:, jj * 9:(jj + 1) * 9] = sl
                jj += 1
        in_maps.append({"coef": coef, "basis": basis, "tex": tex, "scal": scal})

    nc = _build_program(pattern, mhat)
    global LAST_RESULT
    if TRACE:
        res = run_bass_kernel_spmd(nc, in_maps, core_ids=list(range(NCORES)),
                                   trace=True)
    else:
        res = run_bass_kernel_spmd(nc, in_maps, core_ids=list(range(NCORES)))
    LAST_RESULT = res

    out = np.zeros((1, 4, H, W), np.float32)
    for c in range(NCORES):
        o6 = res.results[c]["out6"]                        # [6, NPOS*TP]
        for pos in range(NPOS):
            b, _ = assign[c][pos]
            yb, xb = b // 2, b % 2
            seg = o6[:, pos * TP:(pos + 1) * TP]
            wbg = np.float32(np.exp((EPS - mhat) / GAMMA))
            dsum = seg[3] + wbg
            rgb = seg[0:3] / dsum[None]
            alpha = 1.0 - np.exp(seg[4])
            ys = slice(4 * yb, 4 * yb + 4)
            xs = slice(128 * xb, 128 * xb + 128)
            out[0, 0:3, ys, xs] = rgb.reshape(3, 4, 128)
            out[0, 3, ys, xs] = alpha.reshape(4, 128)
    return out



# revision 20
# speedup vs baseline: 1.1513x; 1.1513x over previous
"""SoftRas-style soft rasterizer on 8 Trainium2 NeuronCores.

Strategy (v2):
- Per-(face,pixel) affine quantities (along-edge coordinate U_e, squared
  anchor distance pa2_e, barycentrics w0/w1/w2) from TensorE matmuls against
  the pixel basis [1, px, py, px^2+py^2] (bf16 3-way-split x 6 combos for
  fp32-class precision at full f32r rate).
- Segment distance^2 per edge in ONE fused custom-DVE op:
      d2_e = max(pa2 - tc*(2U - tc), eps), tc = clip(U, 0, L).
- Only Ln/Exp LUTs on the scalar engine (sqrt via exp(0.5*ln), sigmoid via
  exp + custom 1-NR reciprocal) -> a single activation table, zero reloads.
- dsum folded into the rgb matmul via a 4th all-ones texture column.
- Work split across DVE / Pool / Scalar / PE to balance engine time.
- Host: per-face coefficient prep, per-tile face culling, load balancing,
  final divide + alpha exponentiation (same layout as v1).
"""
import sys
sys.path.insert(0, '/opt/trn_rl_repo')
import numpy as np
import ml_dtypes
from contextlib import ExitStack

import concourse.bass as bass
import concourse.bacc as bacc
import concourse.tile as tile
import concourse.mybir as mybir
from concourse.bass_utils import run_bass_kernel_spmd

TRACE = False
LAST_RESULT = None

F_TOT = 512
H = W = 256
NCORES = 8
TP = 512              # pixels per tile position (4 rows x 128 px)
NPOS = (H * W) // (NCORES * TP)   # 16 tile positions per core
SIGMA = 1e-2
GAMMA = 1e-3
EPS = 1e-3
NEAR, FAR = 1.0, 100.0
FP = mybir.dt.float32
F32R = mybir.dt.float32r
BF = mybir.dt.bfloat16
U32 = mybir.dt.uint32
AL = mybir.AluOpType
AF = mybir.ActivationFunctionType

D2CLAMP = 0.74        # caps dist/sigma at 86 (fp32 underflow floor; matches ref)
RFC0 = -0.23549792    # recip seed Chebyshev consts
RFC1 = 2.0017324


# ---------------------------------------------------------------- custom DVE
def _register_custom_ops():
    """Register the fused DVE ops via the documented OPS extension point."""
    import concourse.dve_ops as dve_ops
    from concourse.dve_ops import (DveOp, OPS, CUSTOM_DVE_SPECS,
                                   _SUB_OPCODE_FOR_NAME, _CUSTOM_DVE_ROW_BASE)
    from concourse.dve_spec import (Spec, Src0, Src1, C0, C1, One, AluOp,
                                    Bin, relu, maxx, minn, lower)
    from concourse.dve_uop import DveOpSpec

    def reg(op_name, spec):
        if op_name in _SUB_OPCODE_FOR_NAME:
            return next(o for o in OPS if o.name == op_name)
        row = _CUSTOM_DVE_ROW_BASE + len(OPS)
        assert row < 0x20
        shas = {}
        for ver in ("v3", "v4"):
            s = DveOpSpec(name=op_name, opcode=row,
                          uops=lower(spec, ver=ver), rd1_en=True)
            shas[ver] = s.sha(ver)
        op = DveOp(op_name, spec, subdim=False, uops_sha=shas)
        OPS.append(op)
        _SUB_OPCODE_FOR_NAME[op_name] = row
        CUSTOM_DVE_SPECS[op_name] = spec
        return op

    def _clip(v, hi):
        return np.minimum(np.maximum(v, 0), hi)

    _tc = minn(relu(Src0), C0)
    edge = reg("ANT_EDGE_M", Spec(
        body=_tc * (Src0 + Src0 - _tc),
        reference=lambda in0, in1, s0, s1, imm2: (
            _clip(in0, s0) * (2 * in0 - _clip(in0, s0))
        ).astype(np.float32)))

    _xp = One + Src0
    _nx = Bin(AluOp.BITWISE_NOT, _xp, _xp)
    _y0 = _nx * C0

    def _r1p_ref(in0, in1, s0, s1, imm2):
        xp = (1.0 + in0).astype(np.float32)
        y0 = ((~xp.view(np.int32)).view(np.float32) * np.float32(s0)).astype(np.float32)
        return (y0 * (s1 - xp * y0)).astype(np.float32)

    r1p = reg("ANT_RECIP1P_1NR",
              Spec(body=_y0 * (C1 - _xp * _y0), reference=_r1p_ref))

    _nx2 = Bin(AluOp.BITWISE_NOT, Src0, Src0)
    _y02 = _nx2 * C0

    def _rm_ref(in0, in1, s0, s1, imm2):
        y0 = ((~in0.view(np.int32)).view(np.float32) * np.float32(s0)).astype(np.float32)
        return ((y0 * (s1 - in0 * y0)) * in1).astype(np.float32)

    rm = reg("ANT_RECIP_MUL_1NR",
             Spec(body=(_y02 * (C1 - Src0 * _y02)) * Src1, reference=_rm_ref))
    return edge, r1p, rm


# ------------------------------------------------------------------ host prep
def _host_prep(face_vertices):
    """Per-face coefficients in basis [1, px, py, r2], fp64."""
    fv = np.asarray(face_vertices, np.float64)[0]          # [F,3,3]
    F = fv.shape[0]
    x = fv[:, :, 0]; y = fv[:, :, 1]; z = fv[:, :, 2]
    x0, x1, x2 = x[:, 0], x[:, 1], x[:, 2]
    y0, y1, y2 = y[:, 0], y[:, 1], y[:, 2]

    den = (y1 - y2) * (x0 - x2) + (x2 - x1) * (y0 - y2)
    den = np.where(np.abs(den) < 1e-10, 1e-10, den)
    W0c = np.stack([(-(y1 - y2) * x2 - (x2 - x1) * y2) / den,
                    (y1 - y2) / den, (x2 - x1) / den, np.zeros(F)], -1)
    W1c = np.stack([(-(y2 - y0) * x2 - (x0 - x2) * y2) / den,
                    (y2 - y0) / den, (x0 - x2) / den, np.zeros(F)], -1)

    anchors = [(x0, y0), (x1, y1), (x2, y2)]
    pairs = [(0, 1), (1, 2), (2, 0)]
    # per edge: U = (p-a).d/|d| (along-line, length units), pa2 = |p-a|^2
    UT = np.zeros((3, F, 4)); PA = np.zeros((3, F, 4)); LL = np.zeros((3, F))
    for e, (ia, ib) in enumerate(pairs):
        ax, ay = anchors[ia]; bx, by = anchors[ib]
        dx, dy = bx - ax, by - ay
        L = np.sqrt(np.maximum(dx * dx + dy * dy, 1e-12))
        iL = 1.0 / L
        UT[e, :, 0] = (-ax * dx - ay * dy) * iL
        UT[e, :, 1] = dx * iL
        UT[e, :, 2] = dy * iL
        PA[e, :, 0] = ax * ax + ay * ay
        PA[e, :, 1] = -2.0 * ax
        PA[e, :, 2] = -2.0 * ay
        PA[e, :, 3] = 1.0
        LL[e] = L
    iz = 1.0 / z
    zmin = z.min(1); zmax = z.max(1)
    assert z.min() > NEAR + 0.05 and z.max() < FAR - 0.05, \
        "kernel fast path assumes all vertex depths strictly inside (NEAR,FAR)"
    return dict(W0c=W0c, W1c=W1c, UT=UT, PA=PA, LL=LL, iz=iz,
                ymin=y.min(1), ymax=y.max(1), xmin=x.min(1), xmax=x.max(1),
                zmin=zmin, zmax=zmax)


def _cull_and_balance(prep):
    """Per tile (4 rows x 128 px), the kept-face list; balanced so all cores
    share one chunk-count pattern."""
    nyb = H // 4
    pixc = ((np.arange(H) + 0.5) / H) * 2.0 - 1.0
    tiles = []
    for yb in range(nyb):
        for xb in range(2):
            tiles.append((pixc[4 * yb], pixc[4 * yb + 3],
                          pixc[128 * xb], pixc[128 * xb + 127]))
    tiles = np.array(tiles)                                # [nb, 4]
    ygap = np.maximum(0.0, np.maximum(
        prep['ymin'][None, :] - tiles[:, 1:2],
        tiles[:, 0:1] - prep['ymax'][None, :]))
    xgap = np.maximum(0.0, np.maximum(
        prep['xmin'][None, :] - tiles[:, 3:4],
        tiles[:, 2:3] - prep['xmax'][None, :]))
    gap = np.sqrt(xgap ** 2 + ygap ** 2)
    znUB = (FAR - prep['zmin']) / (FAR - NEAR)
    znLB = (FAR - prep['zmax']) / (FAR - NEAR)
    D = znLB.max()
    MH = znUB.max()
    # every pixel's true zmax lies in [D, MH]; narrow window -> a single
    # global softmax shift MH is exact
    assert MH - D <= 0.07, "global-shift fast path needs a narrow zmax window"
    keep = (gap < 0.17) | ((gap / SIGMA) + (D - znUB) / GAMMA < 87.0)  # [nb,F]

    counts = np.maximum(1, np.ceil(keep.sum(1) / 128).astype(int))
    order = np.argsort(-counts, kind='stable')             # bands, desc count
    pattern = [int(counts[order[p * NCORES]]) for p in range(NPOS)]
    assign = [[None] * NPOS for _ in range(NCORES)]
    for p in range(NPOS):
        for c in range(NCORES):
            b = int(order[p * NCORES + c])
            faces = np.nonzero(keep[b])[0]
            pad = pattern[p] * 128 - len(faces)
            assert pad >= 0
            faces = np.concatenate([faces, -np.ones(pad, np.int64)])
            assign[c][p] = (b, faces)
    return pattern, assign, float(max(MH, EPS))


# 3-way bf16 split: x = h + m + l; 6 combos give fp32-class precision on the
# PE's f32r path.
COMBOS = [(0, 0), (0, 1), (1, 0), (0, 2), (1, 1), (2, 0)]
NK = 4 * len(COMBOS)


def _split3(a):
    a = np.asarray(a, np.float64)
    h = a.astype(ml_dtypes.bfloat16).astype(np.float64)
    r = a - h
    m = r.astype(ml_dtypes.bfloat16).astype(np.float64)
    l = (r - m).astype(ml_dtypes.bfloat16).astype(np.float64)
    return [h, m, l]


def _face_arrays(prep, textures, faces):
    """Coefficient/texture/scalar arrays for one chunk of 128 face slots
    (index -1 = inert dummy). Quantity order:
    U0,pa0,U1,pa1,U2,pa2,W0,W1,W2 -> coef[4, 9, 128]."""
    f = np.asarray(faces)
    dummy = f < 0
    fi = np.where(dummy, 0, f)

    def D(a):
        a = np.asarray(a, np.float64).copy()
        a[dummy] = 0.0
        return a

    coef = np.zeros((4, 9, 128))
    for e in range(3):
        coef[:, 2 * e, :] = D(prep['UT'][e][fi]).T
        coef[:, 2 * e + 1, :] = D(prep['PA'][e][fi]).T
    coef[:, 6, :] = D(prep['W0c'][fi]).T
    coef[:, 7, :] = D(prep['W1c'][fi]).T
    # dummies: pa2 const 10 -> d2 clamps to D2CLAMP -> y=80 -> p ~ 0;
    # W0=W1=-1 (outside), W2=3 -> wc=(0,0,1), s=1
    coef[0, 1, dummy] = 10.0
    coef[0, 3, dummy] = 10.0
    coef[0, 5, dummy] = 10.0
    coef[0, 6, dummy] = -1.0
    coef[0, 7, dummy] = -1.0
    coef[:, 8, :] = -coef[:, 6, :] - coef[:, 7, :]
    coef[0, 8, :] += 1.0                                   # w2 = 1 - w0 - w1
    cs = _split3(coef)
    coefk = np.zeros((NK, 9, 128), np.float32)
    for t, (ci, bi) in enumerate(COMBOS):
        coefk[4 * t:4 * t + 4] = cs[ci].astype(np.float32)

    # tex4: [128, 3k, 4c] = (r,g,b,1); dummies all-zero
    tex4 = np.zeros((128, 3, 4), np.float32)
    tex4[:, :, 0:3] = np.asarray(textures, np.float64)[0][fi]
    tex4[:, :, 3] = 1.0
    tex4[dummy] = 0.0

    # scal: iz0,iz1,iz2, L0,L1,L2
    scal = np.zeros((128, 6))
    izf = prep['iz'][fi]
    izf[dummy] = 0.011
    scal[:, 0:3] = izf
    llf = prep['LL'][:, fi].T
    llf[dummy] = 0.5
    scal[:, 3:6] = llf
    return coefk, tex4, scal


# ------------------------------------------------------------------- program
def _build_program(pattern, mhat):
    EDGE_OP, R1P_OP, RM_OP = _register_custom_ops()
    totc = sum(pattern)
    nc = bacc.Bacc("TRN2", target_bir_lowering=False, debug=False,
                   num_devices=NCORES)
    d_coef = nc.dram_tensor("coef", [totc, NK, 9 * 128], F32R, kind="ExternalInput")
    d_basis = nc.dram_tensor("basis", [NPOS, NK, TP], F32R, kind="ExternalInput")
    d_tex = nc.dram_tensor("tex", [128, totc * 12], BF, kind="ExternalInput")
    d_scal = nc.dram_tensor("scal", [128, totc * 6], FP, kind="ExternalInput")
    d_out = nc.dram_tensor("out6", [5, NPOS * TP], FP, kind="ExternalOutput")

    AA = -1000.0 / 99.0
    BB = (100.0 / 99.0 - mhat) * 1000.0

    with ExitStack() as ctx:
        tc = ctx.enter_context(tile.TileContext(nc))
        const = ctx.enter_context(tc.tile_pool(name="const", bufs=1))
        stage = ctx.enter_context(tc.tile_pool(name="stage", bufs=3))
        basp = ctx.enter_context(tc.tile_pool(name="basp", bufs=3))
        work = ctx.enter_context(tc.tile_pool(name="work", bufs=2))
        outp = ctx.enter_context(tc.tile_pool(name="outp", bufs=3))
        qp = ctx.enter_context(tc.tile_pool(name="qp", bufs=6, space="PSUM"))
        accp = ctx.enter_context(tc.tile_pool(name="accp", bufs=2, space="PSUM"))

        onesb = const.tile([128, 1], BF)
        nc.vector.memset(onesb, 1.0)
        b_y = const.tile([128, 1], FP)
        nc.vector.memset(b_y, float(np.log(1.0 / SIGMA)))
        b_expw = const.tile([128, 1], FP)
        nc.vector.memset(b_expw, BB)
        b_ln = const.tile([128, 1], FP)
        nc.vector.memset(b_ln, 1e-5)
        tex_sb = const.tile([128, totc * 12], BF)
        nc.sync.dma_start(out=tex_sb, in_=d_tex[:, :])
        scal_sb = const.tile([128, totc * 6], FP)
        nc.sync.dma_start(out=scal_sb, in_=d_scal[:, :])

        base = [0]
        for pcount in pattern:
            base.append(base[-1] + pcount)

        for pos in range(NPOS):
            K = pattern[pos]
            bas = basp.tile([NK, TP], F32R, tag="bas")
            nc.sync.dma_start(out=bas, in_=d_basis[pos, :, :])
            acc = accp.tile([33, TP], FP, tag="acc")

            for j in range(K):
                cj = base[pos] + j
                st = stage.tile([NK, 9 * 128], F32R, tag="st")
                nc.sync.dma_start(out=st, in_=d_coef[cj, :, :])
                sc = lambda i: scal_sb[:, cj * 6 + i: cj * 6 + i + 1]

                # --- edge distances: m_e = tc(2U-tc) fused; d2_e = pa2 - m ---
                d2 = []
                for e in range(3):
                    qU = qp.tile([128, TP], FP, tag="q", name=f"qU{e}")
                    nc.tensor.matmul(qU, st[:, (2 * e) * 128:(2 * e + 1) * 128],
                                     bas, start=True, stop=True)
                    qP = qp.tile([128, TP], FP, tag="q", name=f"qP{e}")
                    nc.tensor.matmul(qP, st[:, (2 * e + 1) * 128:(2 * e + 2) * 128],
                                     bas, start=True, stop=True)
                    me = work.tile([128, TP], FP, tag=f"m_{e}", name=f"m_{e}")
                    nc.vector._custom_dve(EDGE_OP, out=me, in0=qU,
                                          s0=sc(3 + e))
                    de = work.tile([128, TP], FP, tag=f"d2_{e}", name=f"d2_{e}")
                    nc.vector.scalar_tensor_tensor(out=de, in0=me,
                                                   scalar=-1.0, in1=qP,
                                                   op0=AL.mult, op1=AL.add)
                    d2.append(de)
                tmin = work.tile([128, TP], FP, tag="tmin")
                nc.vector.tensor_tensor(out=tmin, in0=d2[0], in1=d2[1],
                                        op=AL.min)
                d2c = work.tile([128, TP], FP, tag="d2c")
                nc.vector.scalar_tensor_tensor(out=d2c, in0=d2[2],
                                               scalar=D2CLAMP, in1=tmin,
                                               op0=AL.min, op1=AL.min)
                ld = work.tile([128, TP], FP, tag="ld")
                nc.scalar.activation(ld, d2c, AF.Ln, bias=b_ln)
                yv = work.tile([128, TP], FP, tag="yv")
                nc.scalar.activation(yv, ld, AF.Exp, scale=0.5, bias=b_y)

                # --- barycentrics ---
                qw = []
                for k in range(3):
                    q = qp.tile([128, TP], FP, tag="q", name=f"qw{k}")
                    nc.tensor.matmul(q, st[:, (6 + k) * 128:(7 + k) * 128],
                                     bas, start=True, stop=True)
                    qw.append(q)
                cw0 = work.tile([128, TP], FP, tag="cw0")
                nc.scalar.activation(cw0, qw[0], AF.Copy)
                mn1 = work.tile([128, TP], FP, tag="mn1")
                nc.vector.tensor_tensor(out=mn1, in0=cw0, in1=qw[1],
                                        op=AL.min)
                mn2 = work.tile([128, TP], FP, tag="mn2")
                nc.vector.tensor_tensor(out=mn2, in0=mn1, in1=qw[2],
                                        op=AL.min)
                sb = work.tile([128, TP], mybir.dt.uint32, tag="sb")
                nc.vector.tensor_scalar(out=sb, in0=mn2.bitcast(U32),
                                        scalar1=0x80000000, scalar2=None,
                                        op0=AL.bitwise_and)
                ys = work.tile([128, TP], FP, tag="ys")
                nc.vector.tensor_tensor(out=ys.bitcast(U32), in0=sb,
                                        in1=yv.bitcast(U32),
                                        op=AL.bitwise_or)
                Es = work.tile([128, TP], FP, tag="Es")
                nc.scalar.activation(Es, ys, AF.Exp, scale=-1.0)

                wc = []
                for k in range(3):
                    w = work.tile([128, TP], FP, tag=f"wc{k}", name=f"wc{k}")
                    nc.vector.tensor_scalar(out=w, in0=qw[k], scalar1=0.0,
                                            scalar2=1.0, op0=AL.max,
                                            op1=AL.min)
                    wc.append(w)

                # r1 = sum wc_k iz_k  (Pool), s = sum wc_k (Pool)
                r1a = work.tile([128, TP], FP, tag="r1a")
                nc.vector.tensor_scalar(out=r1a, in0=wc[0], scalar1=sc(0),
                                        scalar2=None, op0=AL.mult)
                r1b = work.tile([128, TP], FP, tag="r1b")
                nc.vector.scalar_tensor_tensor(out=r1b, in0=wc[1],
                                               scalar=sc(1), in1=r1a,
                                               op0=AL.mult, op1=AL.add)
                r1 = work.tile([128, TP], FP, tag="r1")
                nc.vector.scalar_tensor_tensor(out=r1, in0=wc[2],
                                               scalar=sc(2), in1=r1b,
                                               op0=AL.mult, op1=AL.add)
                s01 = work.tile([128, TP], FP, tag="s01")
                nc.gpsimd.tensor_tensor(out=s01, in0=wc[0], in1=wc[1],
                                        op=AL.add)
                ssum = work.tile([128, TP], FP, tag="ssum")
                nc.gpsimd.tensor_tensor(out=ssum, in0=s01, in1=wc[2],
                                        op=AL.add)

                rinv = work.tile([128, TP], FP, tag="rinv")
                nc.vector.reciprocal_approx_fast(out=rinv, in_=r1)
                zp = work.tile([128, TP], FP, tag="zp")
                nc.gpsimd.tensor_tensor(out=zp, in0=ssum, in1=rinv,
                                        op=AL.mult)
                expw = work.tile([128, TP], FP, tag="expw")
                nc.scalar.activation(expw, zp, AF.Exp, scale=AA, bias=b_expw)

                pv = work.tile([128, TP], FP, tag="pv")
                nc.vector._custom_dve(R1P_OP, out=pv, in0=Es,
                                      s0=RFC0, s1=RFC1)
                mpp = work.tile([128, TP], FP, tag="mpp")
                nc.vector.tensor_scalar(out=mpp, in0=pv, scalar1=1.0,
                                        scalar2=-1e-35, op0=AL.subtract,
                                        op1=AL.min)
                lq = work.tile([128, TP], BF, tag="lq")
                nc.scalar.activation(lq, mpp, AF.Ln, scale=-1.0)
                nc.tensor.matmul(acc[32:33, :], onesb[:, 0:1], lq,
                                 start=(j == 0), stop=(j == K - 1))

                wexp = work.tile([128, TP], FP, tag="wexp")
                nc.gpsimd.tensor_tensor(out=wexp, in0=pv, in1=expw,
                                        op=AL.mult)
                psn = work.tile([128, TP], FP, tag="psn")
                nc.vector._custom_dve(RM_OP, out=psn, in0=ssum, in1=wexp,
                                      s0=RFC0, s1=RFC1)
                for k in range(3):
                    g = work.tile([128, TP], BF, tag="g", name=f"g{k}",
                                  bufs=3)
                    nc.gpsimd.tensor_tensor(out=g, in0=psn, in1=wc[k],
                                            op=AL.mult)
                    nc.tensor.matmul(
                        acc[0:4, :],
                        tex_sb[:, cj * 12 + k * 4: cj * 12 + (k + 1) * 4],
                        g, start=(j == 0 and k == 0),
                        stop=(j == K - 1 and k == 2))

            o6 = outp.tile([33, TP], FP, tag="o6")
            nc.scalar.activation(o6[0:4, :], acc[0:4, :], AF.Copy)
            nc.scalar.activation(o6[32:33, :], acc[32:33, :], AF.Copy)
            nc.sync.dma_start(out=d_out[0:4, pos * TP:(pos + 1) * TP],
                              in_=o6[0:4, :])
            nc.sync.dma_start(out=d_out[4:5, pos * TP:(pos + 1) * TP],
                              in_=o6[32:33, :])
    nc.compile()
    return nc


def kernel(face_vertices, face_textures):
    prep = _host_prep(face_vertices)
    pattern, assign, mhat = _cull_and_balance(prep)
    totc = sum(pattern)

    pix = ((np.arange(H, dtype=np.float64) + 0.5) / H) * 2.0 - 1.0
    in_maps = []
    for c in range(NCORES):
        coef = np.zeros((totc, NK, 9 * 128), np.float32)
        tex = np.zeros((128, totc * 12), ml_dtypes.bfloat16)
        scal = np.zeros((128, totc * 6), np.float32)
        basis = np.zeros((NPOS, NK, TP), np.float32)
        jj = 0
        for pos in range(NPOS):
            b, faces = assign[c][pos]
            yb, xb = b // 2, b % 2
            py = np.repeat(pix[4 * yb:4 * yb + 4], 128)
            px = np.tile(pix[128 * xb:128 * xb + 128], 4)
            b4 = np.stack([np.ones(TP), px, py, px ** 2 + py ** 2])
            bs = _split3(b4)
            for t, (ci, bi) in enumerate(COMBOS):
                basis[pos, 4 * t:4 * t + 4] = bs[bi].astype(np.float32)
            for j in range(pattern[pos]):
                cf, tx, sl = _face_arrays(prep, face_textures,
                                          faces[j * 128:(j + 1) * 128])
                coef[jj] = cf.reshape(NK, 9 * 128)
                tex[:, jj * 12:(jj + 1) * 12] = tx.reshape(128, 12)
                scal[:, jj * 6:(jj + 1) * 6] = sl
                jj += 1
        in_maps.append({"coef": coef, "basis": basis, "tex": tex, "scal": scal})

    nc = _build_program(pattern, mhat)
    global LAST_RESULT
    if TRACE:
        res = run_bass_kernel_spmd(nc, in_maps, core_ids=list(range(NCORES)),
                                   trace=True)
    else:
        res = run_bass_kernel_spmd(nc, in_maps, core_ids=list(range(NCORES)))
    LAST_RESULT = res

    out = np.zeros((1, 4, H, W), np.float32)
    wbg = np.float32(np.exp((EPS - mhat) / GAMMA))
    for c in range(NCORES):
        o6 = res.results[c]["out6"]                        # [5, NPOS*TP]
        for pos in range(NPOS):
            b, _ = assign[c][pos]
            yb, xb = b // 2, b % 2
            seg = o6[:, pos * TP:(pos + 1) * TP]
            dsum = seg[3] + wbg
            rgb = seg[0:3] / dsum[None]
            alpha = 1.0 - np.exp(seg[4])
            ys = slice(4 * yb, 4 * yb + 4)
            xs = slice(128 * xb, 128 * xb + 128)
            out[0, 0:3, ys, xs] = rgb.reshape(3, 4, 128)
            out[0, 3, ys, xs] = alpha.reshape(4, 128)
    return out


# revision 23
# speedup vs baseline: 1.4234x; 1.2363x over previous
"""SoftRas-style soft rasterizer on 8 Trainium2 NeuronCores.

Strategy (v2):
- Per-(face,pixel) affine quantities (along-edge coordinate U_e, squared
  anchor distance pa2_e, barycentrics w0/w1/w2) from TensorE matmuls against
  the pixel basis [1, px, py, px^2+py^2] (bf16 3-way-split x 6 combos for
  fp32-class precision at full f32r rate).
- Segment distance^2 per edge in ONE fused custom-DVE op:
      d2_e = max(pa2 - tc*(2U - tc), eps), tc = clip(U, 0, L).
- Only Ln/Exp LUTs on the scalar engine (sqrt via exp(0.5*ln), sigmoid via
  exp + custom 1-NR reciprocal) -> a single activation table, zero reloads.
- dsum folded into the rgb matmul via a 4th all-ones texture column.
- Work split across DVE / Pool / Scalar / PE to balance engine time.
- Host: per-face coefficient prep, per-tile face culling, load balancing,
  final divide + alpha exponentiation (same layout as v1).
"""
import sys
sys.path.insert(0, '/opt/trn_rl_repo')
import numpy as np
import ml_dtypes
from contextlib import ExitStack

import concourse.bass as bass
import concourse.bacc as bacc
import concourse.tile as tile
import concourse.mybir as mybir
from concourse.bass_utils import run_bass_kernel_spmd

TRACE = False
LAST_RESULT = None

F_TOT = 512
H = W = 256
NCORES = 8
TP = 512              # pixels per tile position (4 rows x 128 px)
NPOS = (H * W) // (NCORES * TP)   # 16 tile positions per core
SIGMA = 1e-2
GAMMA = 1e-3
EPS = 1e-3
NEAR, FAR = 1.0, 100.0
FP = mybir.dt.float32
F32R = mybir.dt.float32r
BF = mybir.dt.bfloat16
U32 = mybir.dt.uint32
AL = mybir.AluOpType
AF = mybir.ActivationFunctionType

D2CLAMP = 0.74        # caps dist/sigma at 86 (fp32 underflow floor; matches ref)
RFC0 = -0.23549792    # recip seed Chebyshev consts
RFC1 = 2.0017324


# ---------------------------------------------------------------- custom DVE
def _register_custom_ops():
    """Register the fused DVE ops via the documented OPS extension point."""
    import concourse.dve_ops as dve_ops
    from concourse.dve_ops import (DveOp, OPS, CUSTOM_DVE_SPECS,
                                   _SUB_OPCODE_FOR_NAME, _CUSTOM_DVE_ROW_BASE)
    from concourse.dve_spec import (Spec, Src0, Src1, C0, C1, One, AluOp,
                                    Bin, relu, maxx, minn, lower)
    from concourse.dve_uop import DveOpSpec

    def reg(op_name, spec):
        if op_name in _SUB_OPCODE_FOR_NAME:
            return next(o for o in OPS if o.name == op_name)
        row = _CUSTOM_DVE_ROW_BASE + len(OPS)
        assert row < 0x20
        shas = {}
        for ver in ("v3", "v4"):
            s = DveOpSpec(name=op_name, opcode=row,
                          uops=lower(spec, ver=ver), rd1_en=True)
            shas[ver] = s.sha(ver)
        op = DveOp(op_name, spec, subdim=False, uops_sha=shas)
        OPS.append(op)
        _SUB_OPCODE_FOR_NAME[op_name] = row
        CUSTOM_DVE_SPECS[op_name] = spec
        return op

    def _clip(v, hi):
        return np.minimum(np.maximum(v, 0), hi)

    _tc = minn(relu(Src0), C0)
    edge = reg("ANT_EDGE_M", Spec(
        body=_tc * (Src0 + Src0 - _tc),
        reference=lambda in0, in1, s0, s1, imm2: (
            _clip(in0, s0) * (2 * in0 - _clip(in0, s0))
        ).astype(np.float32)))

    _xp = One + Src0
    _nx = Bin(AluOp.BITWISE_NOT, _xp, _xp)
    _y0 = _nx * C0

    def _r1p_ref(in0, in1, s0, s1, imm2):
        xp = (1.0 + in0).astype(np.float32)
        y0 = ((~xp.view(np.int32)).view(np.float32) * np.float32(s0)).astype(np.float32)
        return (y0 * (s1 - xp * y0)).astype(np.float32)

    r1p = reg("ANT_RECIP1P_1NR",
              Spec(body=_y0 * (C1 - _xp * _y0), reference=_r1p_ref))

    _nx2 = Bin(AluOp.BITWISE_NOT, Src0, Src0)
    _y02 = _nx2 * C0

    def _rm_ref(in0, in1, s0, s1, imm2):
        y0 = ((~in0.view(np.int32)).view(np.float32) * np.float32(s0)).astype(np.float32)
        return ((y0 * (s1 - in0 * y0)) * in1).astype(np.float32)

    rm = reg("ANT_RECIP_MUL_1NR",
             Spec(body=(_y02 * (C1 - Src0 * _y02)) * Src1, reference=_rm_ref))
    return edge, r1p, rm


# ------------------------------------------------------------------ host prep
def _host_prep(face_vertices):
    """Per-face coefficients in basis [1, px, py, r2], fp64."""
    fv = np.asarray(face_vertices, np.float64)[0]          # [F,3,3]
    F = fv.shape[0]
    x = fv[:, :, 0]; y = fv[:, :, 1]; z = fv[:, :, 2]
    x0, x1, x2 = x[:, 0], x[:, 1], x[:, 2]
    y0, y1, y2 = y[:, 0], y[:, 1], y[:, 2]

    den = (y1 - y2) * (x0 - x2) + (x2 - x1) * (y0 - y2)
    den = np.where(np.abs(den) < 1e-10, 1e-10, den)
    W0c = np.stack([(-(y1 - y2) * x2 - (x2 - x1) * y2) / den,
                    (y1 - y2) / den, (x2 - x1) / den, np.zeros(F)], -1)
    W1c = np.stack([(-(y2 - y0) * x2 - (x0 - x2) * y2) / den,
                    (y2 - y0) / den, (x0 - x2) / den, np.zeros(F)], -1)

    anchors = [(x0, y0), (x1, y1), (x2, y2)]
    pairs = [(0, 1), (1, 2), (2, 0)]
    # per edge: U = (p-a).d/|d| (along-line, length units), pa2 = |p-a|^2
    UT = np.zeros((3, F, 4)); PA = np.zeros((3, F, 4)); LL = np.zeros((3, F))
    for e, (ia, ib) in enumerate(pairs):
        ax, ay = anchors[ia]; bx, by = anchors[ib]
        dx, dy = bx - ax, by - ay
        L = np.sqrt(np.maximum(dx * dx + dy * dy, 1e-12))
        iL = 1.0 / L
        UT[e, :, 0] = (-ax * dx - ay * dy) * iL
        UT[e, :, 1] = dx * iL
        UT[e, :, 2] = dy * iL
        PA[e, :, 0] = ax * ax + ay * ay
        PA[e, :, 1] = -2.0 * ax
        PA[e, :, 2] = -2.0 * ay
        PA[e, :, 3] = 1.0
        LL[e] = L
    iz = 1.0 / z
    zmin = z.min(1); zmax = z.max(1)
    assert z.min() > NEAR + 0.05 and z.max() < FAR - 0.05, \
        "kernel fast path assumes all vertex depths strictly inside (NEAR,FAR)"
    return dict(W0c=W0c, W1c=W1c, UT=UT, PA=PA, LL=LL, iz=iz,
                ymin=y.min(1), ymax=y.max(1), xmin=x.min(1), xmax=x.max(1),
                zmin=zmin, zmax=zmax)


def _cull_and_balance(prep):
    """Per tile (4 rows x 128 px), the kept-face list; balanced so all cores
    share one chunk-count pattern."""
    nyb = H // 4
    pixc = ((np.arange(H) + 0.5) / H) * 2.0 - 1.0
    tiles = []
    for yb in range(nyb):
        for xb in range(2):
            tiles.append((pixc[4 * yb], pixc[4 * yb + 3],
                          pixc[128 * xb], pixc[128 * xb + 127]))
    tiles = np.array(tiles)                                # [nb, 4]
    ygap = np.maximum(0.0, np.maximum(
        prep['ymin'][None, :] - tiles[:, 1:2],
        tiles[:, 0:1] - prep['ymax'][None, :]))
    xgap = np.maximum(0.0, np.maximum(
        prep['xmin'][None, :] - tiles[:, 3:4],
        tiles[:, 2:3] - prep['xmax'][None, :]))
    gap = np.sqrt(xgap ** 2 + ygap ** 2)
    znUB = (FAR - prep['zmin']) / (FAR - NEAR)
    znLB = (FAR - prep['zmax']) / (FAR - NEAR)
    D = znLB.max()
    MH = znUB.max()
    # every pixel's true zmax lies in [D, MH]; narrow window -> a single
    # global softmax shift MH is exact
    assert MH - D <= 0.07, "global-shift fast path needs a narrow zmax window"
    keep = (gap < 0.17) | ((gap / SIGMA) + (D - znUB) / GAMMA < 87.0)  # [nb,F]

    counts = np.maximum(1, np.ceil(keep.sum(1) / 128).astype(int))
    order = np.argsort(-counts, kind='stable')             # bands, desc count
    pattern = [int(counts[order[p * NCORES]]) for p in range(NPOS)]
    assign = [[None] * NPOS for _ in range(NCORES)]
    for p in range(NPOS):
        for c in range(NCORES):
            b = int(order[p * NCORES + c])
            faces = np.nonzero(keep[b])[0]
            pad = pattern[p] * 128 - len(faces)
            assert pad >= 0
            faces = np.concatenate([faces, -np.ones(pad, np.int64)])
            assign[c][p] = (b, faces)
    return pattern, assign, float(max(MH, EPS))


# 3-way bf16 split: x = h + m + l; 6 combos give fp32-class precision on the
# PE's f32r path.
COMBOS = [(0, 0), (0, 1), (1, 0), (0, 2), (1, 1), (2, 0)]
NK = 4 * len(COMBOS)


def _split3(a):
    a = np.asarray(a, np.float64)
    h = a.astype(ml_dtypes.bfloat16).astype(np.float64)
    r = a - h
    m = r.astype(ml_dtypes.bfloat16).astype(np.float64)
    l = (r - m).astype(ml_dtypes.bfloat16).astype(np.float64)
    return [h, m, l]


def _face_arrays(prep, textures, faces):
    """Coefficient/texture/scalar arrays for one chunk of 128 face slots
    (index -1 = inert dummy). Quantity order:
    U0,pa0,U1,pa1,U2,pa2,W0,W1,W2 -> coef[4, 9, 128]."""
    f = np.asarray(faces)
    dummy = f < 0
    fi = np.where(dummy, 0, f)

    def D(a):
        a = np.asarray(a, np.float64).copy()
        a[dummy] = 0.0
        return a

    coef = np.zeros((4, 9, 128))
    for e in range(3):
        coef[:, 2 * e, :] = D(prep['UT'][e][fi]).T
        coef[:, 2 * e + 1, :] = D(prep['PA'][e][fi]).T
    coef[:, 6, :] = D(prep['W0c'][fi]).T
    coef[:, 7, :] = D(prep['W1c'][fi]).T
    # dummies: pa2 const 10 -> d2 clamps to D2CLAMP -> y=80 -> p ~ 0;
    # W0=W1=-1 (outside), W2=3 -> wc=(0,0,1), s=1
    coef[0, 1, dummy] = 10.0
    coef[0, 3, dummy] = 10.0
    coef[0, 5, dummy] = 10.0
    coef[0, 6, dummy] = -1.0
    coef[0, 7, dummy] = -1.0
    coef[:, 8, :] = -coef[:, 6, :] - coef[:, 7, :]
    coef[0, 8, :] += 1.0                                   # w2 = 1 - w0 - w1
    cs = _split3(coef)
    coefk = np.zeros((NK, 9, 128), np.float32)
    for t, (ci, bi) in enumerate(COMBOS):
        coefk[4 * t:4 * t + 4] = cs[ci].astype(np.float32)

    # tex4: [128, 3k, 4c] = (r,g,b,1); dummies all-zero
    tex4 = np.zeros((128, 3, 4), np.float32)
    tex4[:, :, 0:3] = np.asarray(textures, np.float64)[0][fi]
    tex4[:, :, 3] = 1.0
    tex4[dummy] = 0.0

    # scal: iz0,iz1,iz2, L0,L1,L2
    scal = np.zeros((128, 6))
    izf = prep['iz'][fi]
    izf[dummy] = 0.011
    scal[:, 0:3] = izf
    llf = prep['LL'][:, fi].T
    llf[dummy] = 0.5
    scal[:, 3:6] = llf
    return coefk, tex4, scal


# ------------------------------------------------------------------- program
class _OneTableBacc(bacc.Bacc):
    """Steer every Ln/Exp/Copy/Identity activation to the combined
    natural_log_exp_and_others table so the whole kernel needs exactly one
    ACT_TABLE_LOAD (the default greedy choice alternates ln-only/exp-only
    tables, costing ~1.3us per reload)."""

    def insert_act_table_loads(self):
        import concourse.mybir as mb
        from concourse.hw_specs import get_activation_tables
        import bass_rust as _bass_rust
        has_activation = any(
            isinstance(i, mb.InstActivation)
            for b in self.main_func.blocks
            for i in b.instructions
        )
        if not has_activation:
            return
        mine = {mb.ActivationFunctionType.Ln, mb.ActivationFunctionType.Exp,
                mb.ActivationFunctionType.Copy,
                mb.ActivationFunctionType.Identity}
        tables = []
        for name, fns in get_activation_tables(self.m.arch).items():
            if name != "natural_log_exp_and_others":
                fns = fns - mine
            tables.append((name, fns))
        _bass_rust.insert_act_table_loads(self, tables)


def _build_program(pattern, mhat):
    EDGE_OP, R1P_OP, RM_OP = _register_custom_ops()
    totc = sum(pattern)
    nc = _OneTableBacc("TRN2", target_bir_lowering=False, debug=False,
                       num_devices=NCORES)
    d_coef = nc.dram_tensor("coef", [totc, NK, 9 * 128], F32R, kind="ExternalInput")
    d_basis = nc.dram_tensor("basis", [NPOS, NK, TP], F32R, kind="ExternalInput")
    d_tex = nc.dram_tensor("tex", [128, totc * 12], BF, kind="ExternalInput")
    d_scal = nc.dram_tensor("scal", [128, totc * 6], FP, kind="ExternalInput")
    d_out = nc.dram_tensor("out6", [5, NPOS * TP], FP, kind="ExternalOutput")

    AA = -1000.0 / 99.0
    BB = (100.0 / 99.0 - mhat) * 1000.0

    with ExitStack() as ctx:
        tc = ctx.enter_context(tile.TileContext(nc))
        const = ctx.enter_context(tc.tile_pool(name="const", bufs=1))
        stage = ctx.enter_context(tc.tile_pool(name="stage", bufs=3))
        basp = ctx.enter_context(tc.tile_pool(name="basp", bufs=3))
        work = ctx.enter_context(tc.tile_pool(name="work", bufs=2))
        outp = ctx.enter_context(tc.tile_pool(name="outp", bufs=3))
        qp = ctx.enter_context(tc.tile_pool(name="qp", bufs=6, space="PSUM"))
        accp = ctx.enter_context(tc.tile_pool(name="accp", bufs=2, space="PSUM"))

        onesb = const.tile([128, 1], BF)
        nc.vector.memset(onesb, 1.0)
        b_y = const.tile([128, 1], FP)
        nc.vector.memset(b_y, float(np.log(1.0 / SIGMA)))
        b_expw = const.tile([128, 1], FP)
        nc.vector.memset(b_expw, BB)
        b_ln = const.tile([128, 1], FP)
        nc.vector.memset(b_ln, 1e-5)
        tex_sb = const.tile([128, totc * 12], BF)
        nc.sync.dma_start(out=tex_sb, in_=d_tex[:, :])
        scal_sb = const.tile([128, totc * 6], FP)
        nc.sync.dma_start(out=scal_sb, in_=d_scal[:, :])

        base = [0]
        for pcount in pattern:
            base.append(base[-1] + pcount)

        def phase1(pos):
            K = pattern[pos]
            bas = basp.tile([NK, TP], F32R, tag="bas")
            nc.sync.dma_start(out=bas, in_=d_basis[pos, :, :])
            state = []
            for j in range(K):
                cj = base[pos] + j
                st = stage.tile([NK, 9 * 128], F32R, tag="st")
                nc.sync.dma_start(out=st, in_=d_coef[cj, :, :])
                sc = lambda i: scal_sb[:, cj * 6 + i: cj * 6 + i + 1]

                # edge distances: m_e = tc(2U-tc) fused; d2_e = pa2 - m
                d2 = []
                for e in range(3):
                    qU = qp.tile([128, TP], FP, tag="q", name=f"qU{e}")
                    nc.tensor.matmul(qU, st[:, (2 * e) * 128:(2 * e + 1) * 128],
                                     bas, start=True, stop=True)
                    qP = qp.tile([128, TP], FP, tag="q", name=f"qP{e}")
                    nc.tensor.matmul(qP, st[:, (2 * e + 1) * 128:(2 * e + 2) * 128],
                                     bas, start=True, stop=True)
                    me = work.tile([128, TP], FP, tag=f"m_{e}", name=f"m_{e}")
                    nc.vector._custom_dve(EDGE_OP, out=me, in0=qU,
                                          s0=sc(3 + e))
                    de = work.tile([128, TP], FP, tag=f"d2_{e}", name=f"d2_{e}")
                    nc.vector.scalar_tensor_tensor(out=de, in0=me,
                                                   scalar=-1.0, in1=qP,
                                                   op0=AL.mult, op1=AL.add)
                    d2.append(de)
                tmin = work.tile([128, TP], FP, tag="tmin")
                nc.vector.tensor_tensor(out=tmin, in0=d2[0], in1=d2[1],
                                        op=AL.min)
                d2c = work.tile([128, TP], FP, tag="d2c")
                nc.vector.scalar_tensor_tensor(out=d2c, in0=d2[2],
                                               scalar=D2CLAMP, in1=tmin,
                                               op0=AL.min, op1=AL.min)
                ld = work.tile([128, TP], FP, tag="ld")
                nc.scalar.activation(ld, d2c, AF.Ln, bias=b_ln)
                yv = work.tile([128, TP], FP, tag="yv")
                nc.scalar.activation(yv, ld, AF.Exp, scale=0.5, bias=b_y)

                # barycentrics
                qw = []
                for k in range(3):
                    q = qp.tile([128, TP], FP, tag="q", name=f"qw{k}")
                    nc.tensor.matmul(q, st[:, (6 + k) * 128:(7 + k) * 128],
                                     bas, start=True, stop=True)
                    qw.append(q)
                cw0 = work.tile([128, TP], FP, tag="cw0")
                nc.scalar.activation(cw0, qw[0], AF.Copy)
                mn1 = work.tile([128, TP], FP, tag="mn1")
                nc.vector.tensor_tensor(out=mn1, in0=cw0, in1=qw[1],
                                        op=AL.min)
                mn2 = work.tile([128, TP], FP, tag="mn2")
                nc.vector.tensor_tensor(out=mn2, in0=mn1, in1=qw[2],
                                        op=AL.min)
                sb = work.tile([128, TP], mybir.dt.uint32, tag="sb")
                nc.vector.tensor_scalar(out=sb, in0=mn2.bitcast(U32),
                                        scalar1=0x80000000, scalar2=None,
                                        op0=AL.bitwise_and)
                ys = work.tile([128, TP], FP, tag="ys")
                nc.vector.tensor_tensor(out=ys.bitcast(U32), in0=sb,
                                        in1=yv.bitcast(U32),
                                        op=AL.bitwise_or)
                Es = work.tile([128, TP], FP, tag="Es")
                nc.scalar.activation(Es, ys, AF.Exp, scale=-1.0)

                wc = []
                for k in range(3):
                    w = work.tile([128, TP], FP, tag=f"wc{k}", name=f"wc{k}")
                    nc.vector.tensor_scalar(out=w, in0=qw[k], scalar1=0.0,
                                            scalar2=1.0, op0=AL.max,
                                            op1=AL.min)
                    wc.append(w)

                r1a = work.tile([128, TP], FP, tag="r1a")
                nc.vector.tensor_scalar(out=r1a, in0=wc[0], scalar1=sc(0),
                                        scalar2=None, op0=AL.mult)
                r1b = work.tile([128, TP], FP, tag="r1b")
                nc.vector.scalar_tensor_tensor(out=r1b, in0=wc[1],
                                               scalar=sc(1), in1=r1a,
                                               op0=AL.mult, op1=AL.add)
                r1 = work.tile([128, TP], FP, tag="r1")
                nc.vector.scalar_tensor_tensor(out=r1, in0=wc[2],
                                               scalar=sc(2), in1=r1b,
                                               op0=AL.mult, op1=AL.add)
                s01 = work.tile([128, TP], FP, tag="s01")
                nc.gpsimd.tensor_tensor(out=s01, in0=wc[0], in1=wc[1],
                                        op=AL.add)
                ssum = work.tile([128, TP], FP, tag="ssum")
                nc.gpsimd.tensor_tensor(out=ssum, in0=s01, in1=wc[2],
                                        op=AL.add)

                rinv = work.tile([128, TP], FP, tag="rinv")
                nc.vector.reciprocal_approx_fast(out=rinv, in_=r1)
                zp = work.tile([128, TP], FP, tag="zp")
                nc.gpsimd.tensor_tensor(out=zp, in0=ssum, in1=rinv,
                                        op=AL.mult)
                expw = work.tile([128, TP], FP, tag="expw")
                nc.scalar.activation(expw, zp, AF.Exp, scale=AA, bias=b_expw)
                state.append((cj, Es, expw, ssum, wc))
            return state

        def phase2(pos, state):
            K = pattern[pos]
            acc = accp.tile([33, TP], FP, tag="acc")
            for j, (cj, Es, expw, ssum, wc) in enumerate(state):
                pv = work.tile([128, TP], FP, tag="pv")
                nc.vector._custom_dve(R1P_OP, out=pv, in0=Es,
                                      s0=RFC0, s1=RFC1)
                mpp = work.tile([128, TP], FP, tag="mpp")
                nc.vector.tensor_scalar(out=mpp, in0=pv, scalar1=1.0,
                                        scalar2=-1e-35, op0=AL.subtract,
                                        op1=AL.min)
                lq = work.tile([128, TP], BF, tag="lq")
                nc.scalar.activation(lq, mpp, AF.Ln, scale=-1.0)
                nc.tensor.matmul(acc[32:33, :], onesb[:, 0:1], lq,
                                 start=(j == 0), stop=(j == K - 1))

                wexp = work.tile([128, TP], FP, tag="wexp")
                nc.gpsimd.tensor_tensor(out=wexp, in0=pv, in1=expw,
                                        op=AL.mult)
                psn = work.tile([128, TP], FP, tag="psn")
                nc.vector._custom_dve(RM_OP, out=psn, in0=ssum, in1=wexp,
                                      s0=RFC0, s1=RFC1)
                for k in range(3):
                    g = work.tile([128, TP], BF, tag="g", name=f"g{k}",
                                  bufs=3)
                    nc.gpsimd.tensor_tensor(out=g, in0=psn, in1=wc[k],
                                            op=AL.mult)
                    nc.tensor.matmul(
                        acc[0:4, :],
                        tex_sb[:, cj * 12 + k * 4: cj * 12 + (k + 1) * 4],
                        g, start=(j == 0 and k == 0),
                        stop=(j == K - 1 and k == 2))

            o6 = outp.tile([33, TP], FP, tag="o6")
            nc.scalar.activation(o6[0:4, :], acc[0:4, :], AF.Copy)
            nc.scalar.activation(o6[32:33, :], acc[32:33, :], AF.Copy)
            nc.sync.dma_start(out=d_out[0:4, pos * TP:(pos + 1) * TP],
                              in_=o6[0:4, :])
            nc.sync.dma_start(out=d_out[4:5, pos * TP:(pos + 1) * TP],
                              in_=o6[32:33, :])

        pending = None
        for pos in range(NPOS):
            st1 = phase1(pos)
            if pending is not None:
                phase2(*pending)
            pending = (pos, st1)
        phase2(*pending)
    nc.compile()
    return nc


def kernel(face_vertices, face_textures):
    prep = _host_prep(face_vertices)
    pattern, assign, mhat = _cull_and_balance(prep)
    totc = sum(pattern)

    pix = ((np.arange(H, dtype=np.float64) + 0.5) / H) * 2.0 - 1.0
    in_maps = []
    for c in range(NCORES):
        coef = np.zeros((totc, NK, 9 * 128), np.float32)
        tex = np.zeros((128, totc * 12), ml_dtypes.bfloat16)
        scal = np.zeros((128, totc * 6), np.float32)
        basis = np.zeros((NPOS, NK, TP), np.float32)
        jj = 0
        for pos in range(NPOS):
            b, faces = assign[c][pos]
            yb, xb = b // 2, b % 2
            py = np.repeat(pix[4 * yb:4 * yb + 4], 128)
            px = np.tile(pix[128 * xb:128 * xb + 128], 4)
            b4 = np.stack([np.ones(TP), px, py, px ** 2 + py ** 2])
            bs = _split3(b4)
            for t, (ci, bi) in enumerate(COMBOS):
                basis[pos, 4 * t:4 * t + 4] = bs[bi].astype(np.float32)
            for j in range(pattern[pos]):
                cf, tx, sl = _face_arrays(prep, face_textures,
                                          faces[j * 128:(j + 1) * 128])
                coef[jj] = cf.reshape(NK, 9 * 128)
                tex[:, jj * 12:(jj + 1) * 12] = tx.reshape(128, 12)
                scal[:, jj * 6:(jj + 1) * 6] = sl
                jj += 1
        in_maps.append({"coef": coef, "basis": basis, "tex": tex, "scal": scal})

    nc = _build_program(pattern, mhat)
    global LAST_RESULT
    if TRACE:
        res = run_bass_kernel_spmd(nc, in_maps, core_ids=list(range(NCORES)),
                                   trace=True)
    else:
        res = run_bass_kernel_spmd(nc, in_maps, core_ids=list(range(NCORES)))
    LAST_RESULT = res

    out = np.zeros((1, 4, H, W), np.float32)
    wbg = np.float32(np.exp((EPS - mhat) / GAMMA))
    for c in range(NCORES):
        o6 = res.results[c]["out6"]                        # [5, NPOS*TP]
        for pos in range(NPOS):
            b, _ = assign[c][pos]
            yb, xb = b // 2, b % 2
            seg = o6[:, pos * TP:(pos + 1) * TP]
            dsum = seg[3] + wbg
            rgb = seg[0:3] / dsum[None]
            alpha = 1.0 - np.exp(seg[4])
            ys = slice(4 * yb, 4 * yb + 4)
            xs = slice(128 * xb, 128 * xb + 128)
            out[0, 0:3, ys, xs] = rgb.reshape(3, 4, 128)
            out[0, 3, ys, xs] = alpha.reshape(4, 128)
    return out
